# revision 10
# baseline (speedup 1.0000x reference)
"""Chamfer loss kernel for Trainium2 (8 NeuronCores, SPMD).

Math: loss = 10 * 0.5 * (mean(sqrt(dist1)) + mean(sqrt(dist2)))
  dist1[j] = min_i ||target_j - state_x_i||^2   (over all state_x)
  dist2[i] = min_j ||state_x_i - target_j||^2   (over all target)

Device strategy (per core k of 8):
  - i (state_x) is sharded: core k owns rows [2048k, 2048(k+1)).
  - j (target) is replicated (stationary matmul operand).
  - K=5 augmented vectors:  bhat_j = [1, |b|^2, -2bx, -2by, -2bz],
    ahat_i = [|a|^2, 1, ax, ay, az]  so  bhat_j . ahat_i = d(i, j).
  - Per group c (128 target points): PE computes d tile [128 j, 2048 i]
    into PSUM (4 banks, 4x N=512 f32 matmuls).
  - DVE tensor_tensor_reduce: out = max(d, 0) -> fp16 SBUF copy,
    accum_out = min over i  -> exact f32 per-target partial min.
  - DVE tensor_tensor(min) folds the fp16 copy into a running [128, 2048]
    accumulator (the per-state_x min lands in partition-residue form).
  - Host: partition-residue min, cross-core combine, sqrt/mean epilogue.
"""

import os

import numpy as np

N = 16384
N_CORES = 8
I_PER_CORE = N // N_CORES  # 2048 streaming points per core
JC = 128                   # stationary chunk (output partitions per group)
GROUPS = N // JC           # 128 groups per core
FREE = I_PER_CORE          # 2048 free-dim elements per group
MM_N = 512                 # max f32 moving free dim (one PSUM bank)
K = 5

# Fraction of row-accumulate tensor_tensor ops to run on GPSIMD instead of
# DVE (load balancing knob; 0 = all DVE).
GPSIMD_ROWACC = float(os.environ.get("CHAMFER_GPSIMD_ROWACC", "0.0"))

_CACHE = {}

# Results of the last hardware run (BassKernelResults); test harness reads
# this for exec_time_ns when BASS_TRACE=1.
LAST_RESULTS = None


def _get_minred_op():
    """Register (once) a custom DVE op:
        out       = relu(in0)            (clamp + dtype-converting copy)
        accum_out = min(s0, min_k out[k])  (free-axis min reduce)
    One 1x DVE pass fuses the PSUM drain, the clamp, the fp16 copy and the
    free-axis min.  (The native TENSOR_TENSOR_REDUCE ISA opcode is rejected
    by this walrus build, hence the custom-table route.)
    """
    if "minred" in _CACHE:
        return _CACHE["minred"]

    import numpy as np
    from concourse import dve_ops
    from concourse.dve_spec import Spec, Src0, C0, lower, minn, relu, _has_src1
    from concourse.dve_uop import DveOpSpec

    def _ref(in0, in1, c0, c1, c2):
        b = np.maximum(
            np.nan_to_num(
                in0.astype(np.float32), nan=0.0, posinf=np.inf, neginf=-np.inf
            ),
            0.0,
        )
        acc = np.minimum(c0, b.reshape(b.shape[0], -1).min(axis=-1, keepdims=True))
        return b, acc

    spec = Spec(body=relu(Src0), accum=minn, accum_init=C0, reference=_ref)
    op = dve_ops.DveOp("CHAMFER_RELU_MINRED", spec, subdim=False, uops_sha={})
    dve_ops.OPS.append(op)
    dve_ops._SUB_OPCODE_FOR_NAME[op.name] = (
        max(dve_ops._SUB_OPCODE_FOR_NAME.values()) + 1
    )
    dve_ops.CUSTOM_DVE_SPECS[op.name] = op.spec
    for ver in ("v3", "v4"):
        s = DveOpSpec(
            name=op.name,
            opcode=dve_ops.get_dve_sub_opcode(op.name),
            uops=lower(spec, ver=ver),
            rd1_en=_has_src1(spec),
        )
        op.uops_sha[ver] = s.sha(ver)
    _CACHE["minred"] = op
    return op


def _build_nc():
    import concourse.mybir as mybir
    from concourse import bacc
    from concourse.tile import TileContext

    f32 = mybir.dt.float32
    f16 = mybir.dt.float16

    minred = _get_minred_op()
    nc = bacc.Bacc(
        "TRN2",
        target_bir_lowering=False,
        debug=False,
        enable_asserts=True,
        num_devices=N_CORES,
    )

    # One input tensor (single DMA → single wait sem on the first matmul):
    # [:, :I_PER_CORE] = streaming ahat slice, [:, I_PER_CORE:] = full bhat.
    ab_aug = nc.dram_tensor("ab_aug", [K, I_PER_CORE + N], f32, kind="ExternalInput")
    colmin_d = nc.dram_tensor("colmin", [JC, GROUPS], f32, kind="ExternalOutput")
    rowacc_d = nc.dram_tensor("rowacc", [JC, FREE], f16, kind="ExternalOutput")

    n_gp = int(round(GROUPS * GPSIMD_ROWACC))

    with TileContext(nc) as tc:
        with (
            tc.tile_pool(name="const", bufs=1) as const_pool,
            tc.tile_pool(name="copies", bufs=4) as copy_pool,
            tc.tile_pool(name="psum", bufs=2, space="PSUM") as psum_pool,
        ):
            ab_sb = const_pool.tile([K, I_PER_CORE + N], f32)
            nc.sync.dma_start(ab_sb[:], ab_aug[:])
            a_sb = ab_sb[:, :I_PER_CORE]
            b_sb = ab_sb[:, I_PER_CORE:]

            colmin_sb = const_pool.tile([JC, GROUPS], f32)

            racc = [
                const_pool.tile([JC, FREE], f16, name=f"racc{i}") for i in range(2)
            ]
            nc.gpsimd.memset(racc[0][:], 60000.0)

            for c in range(GROUPS):
                pt = psum_pool.tile([JC, FREE], f32, tag="pt")
                for s in range(FREE // MM_N):
                    nc.tensor.matmul(
                        pt[:, s * MM_N : (s + 1) * MM_N],
                        b_sb[:, c * JC : (c + 1) * JC],
                        a_sb[:, s * MM_N : (s + 1) * MM_N],
                        start=True,
                        stop=True,
                    )
                dcopy = copy_pool.tile([JC, FREE], f16, tag="dcopy")
                nc.vector._custom_dve(
                    minred,
                    out=dcopy[:],
                    in0=pt[:],
                    s0=3.0e38,
                    accum_out=colmin_sb[:, c : c + 1],
                )
                # Running min across groups; ping-pong buffers.
                eng = nc.gpsimd if c < n_gp else nc.vector
                eng.tensor_tensor(
                    racc[(c + 1) % 2][:],
                    racc[c % 2][:],
                    dcopy[:],
                    mybir.AluOpType.min,
                )

            nc.sync.dma_start(colmin_d[:], colmin_sb[:])
            nc.sync.dma_start(rowacc_d[:], racc[GROUPS % 2][:])

    nc.compile()
    return nc


def _augment(pts):
    """pts [N, 3] f32 -> (ahat [5, N], bhat [5, N]) f32."""
    pts = np.asarray(pts, dtype=np.float32)
    sq = np.sum(pts.astype(np.float32) * pts, axis=1, dtype=np.float32)
    n = pts.shape[0]
    ahat = np.empty((K, n), dtype=np.float32)
    ahat[0] = sq
    ahat[1] = 1.0
    ahat[2:5] = pts.T
    bhat = np.empty((K, n), dtype=np.float32)
    bhat[0] = 1.0
    bhat[1] = sq
    bhat[2:5] = -2.0 * pts.T
    return ahat, bhat


def kernel(state_x, target):
    global LAST_RESULTS
    from concourse.bass_utils import run_bass_kernel_spmd

    state_x = np.asarray(state_x, dtype=np.float32)
    target = np.asarray(target, dtype=np.float32)

    if "nc" not in _CACHE:
        _CACHE["nc"] = _build_nc()
    nc = _CACHE["nc"]

    ahat, _ = _augment(state_x)   # streaming side: state_x
    _, bhat = _augment(target)    # stationary side: target

    in_maps = []
    for k in range(N_CORES):
        sl = slice(k * I_PER_CORE, (k + 1) * I_PER_CORE)
        ab = np.concatenate([ahat[:, sl], bhat], axis=1)
        in_maps.append({"ab_aug": np.ascontiguousarray(ab)})

    res = run_bass_kernel_spmd(nc, in_maps, core_ids=list(range(N_CORES)))
    LAST_RESULTS = res

    # dist2[i] = min_j d(i, j): partition-residue min of the row accumulator.
    dist2 = np.empty(N, dtype=np.float32)
    # dist1[j] = min_i d(i, j): combine per-core partials.
    dist1 = np.full(N, np.inf, dtype=np.float32)
    for k in range(N_CORES):
        out = res.results[k]
        racc = out["rowacc"].astype(np.float32)       # [128, 2048]
        dist2[k * I_PER_CORE : (k + 1) * I_PER_CORE] = racc.min(axis=0)
        colmin = out["colmin"]                        # [128, 128] [p, c]
        dist1 = np.minimum(dist1, colmin.T.reshape(N))

    dist1 = np.maximum(dist1, 0.0)
    dist2 = np.maximum(dist2, 0.0)
    loss = 0.5 * (np.mean(np.sqrt(dist1), dtype=np.float32)
                  + np.mean(np.sqrt(dist2), dtype=np.float32)) * 10.0
    return np.float32(loss)


# revision 15
# speedup vs baseline: 2.0016x; 2.0016x over previous
"""Chamfer loss kernel for Trainium2 (8 NeuronCores, SPMD).

Math: loss = 10 * 0.5 * (mean(sqrt(dist1)) + mean(sqrt(dist2)))
  dist1[j] = min_i ||target_j - state_x_i||^2   (over all state_x)
  dist2[i] = min_j ||state_x_i - target_j||^2   (over all target)

Device strategy (per core k of 8):
  - i (state_x) is sharded: core k owns rows [2048k, 2048(k+1)).
  - j (target) is replicated (stationary matmul operand).
  - K=5 augmented vectors:  bhat_j = [1, |b|^2, -2bx, -2by, -2bz],
    ahat_i = [|a|^2, 1, ax, ay, az]  so  bhat_j . ahat_i = d(i, j).
  - Per group c (128 target points): PE computes d tile [128 j, 2048 i]
    into PSUM (4 banks, 4x N=512 f32 matmuls).
  - DVE tensor_tensor_reduce: out = max(d, 0) -> fp16 SBUF copy,
    accum_out = min over i  -> exact f32 per-target partial min.
  - DVE tensor_tensor(min) folds the fp16 copy into a running [128, 2048]
    accumulator (the per-state_x min lands in partition-residue form).
  - Host: partition-residue min, cross-core combine, sqrt/mean epilogue.
"""

import os

import numpy as np

N = 16384
N_CORES = 8
I_PER_CORE = N // N_CORES  # 2048 streaming points per core
JC = 128                   # stationary chunk (output partitions per group)
GROUPS = N // JC           # 128 groups per core
FREE = I_PER_CORE          # 2048 free-dim elements per group
MM_N = 512                 # one PSUM bank of f32 output per matmul
K = 5                      # augmented coordinate count
# fp16 hi/lo split: d = a_hi.b_hi + a_lo.b_hi + a_hi.b_lo (error ~2^-21)
KSPLIT = 3 * K             # contraction dim of the fp16 matmul

# Fraction of row-accumulate tensor_tensor ops to run on GPSIMD instead of
# DVE (load balancing knob; 0 = all DVE).
GPSIMD_ROWACC = float(os.environ.get("CHAMFER_GPSIMD_ROWACC", "0.0"))

_CACHE = {}

# Results of the last hardware run (BassKernelResults); test harness reads
# this for exec_time_ns when BASS_TRACE=1.
LAST_RESULTS = None


def _get_minred_op():
    """Register (once) a custom DVE op:
        out       = relu(in0)            (clamp + dtype-converting copy)
        accum_out = min(s0, min_k out[k])  (free-axis min reduce)
    One 1x DVE pass fuses the PSUM drain, the clamp, the fp16 copy and the
    free-axis min.  (The native TENSOR_TENSOR_REDUCE ISA opcode is rejected
    by this walrus build, hence the custom-table route.)
    """
    if "minred" in _CACHE:
        return _CACHE["minred"]

    import numpy as np
    from concourse import dve_ops
    from concourse.dve_spec import Spec, Src0, C0, lower, minn, relu, _has_src1
    from concourse.dve_uop import DveOpSpec

    def _ref(in0, in1, c0, c1, c2):
        b = np.maximum(
            np.nan_to_num(
                in0.astype(np.float32), nan=0.0, posinf=np.inf, neginf=-np.inf
            ),
            0.0,
        )
        acc = np.minimum(c0, b.reshape(b.shape[0], -1).min(axis=-1, keepdims=True))
        return b, acc

    spec = Spec(body=relu(Src0), accum=minn, accum_init=C0, reference=_ref)
    op = dve_ops.DveOp("CHAMFER_RELU_MINRED", spec, subdim=False, uops_sha={})
    dve_ops.OPS.append(op)
    dve_ops._SUB_OPCODE_FOR_NAME[op.name] = (
        max(dve_ops._SUB_OPCODE_FOR_NAME.values()) + 1
    )
    dve_ops.CUSTOM_DVE_SPECS[op.name] = op.spec
    for ver in ("v3", "v4"):
        s = DveOpSpec(
            name=op.name,
            opcode=dve_ops.get_dve_sub_opcode(op.name),
            uops=lower(spec, ver=ver),
            rd1_en=_has_src1(spec),
        )
        op.uops_sha[ver] = s.sha(ver)
    _CACHE["minred"] = op
    return op


def _build_nc():
    import concourse.mybir as mybir
    from concourse import bacc
    from concourse.tile import TileContext

    f32 = mybir.dt.float32
    f16 = mybir.dt.float16

    minred = _get_minred_op()
    nc = bacc.Bacc(
        "TRN2",
        target_bir_lowering=False,
        debug=False,
        enable_asserts=True,
        num_devices=N_CORES,
    )

    # One input tensor (single DMA → single wait sem on the first matmul):
    # [:, :I_PER_CORE] = streaming ahat slice, [:, I_PER_CORE:] = full bhat.
    ab_aug = nc.dram_tensor(
        "ab_aug", [KSPLIT, I_PER_CORE + N], f16, kind="ExternalInput"
    )
    colmin_d = nc.dram_tensor("colmin", [JC, GROUPS], f32, kind="ExternalOutput")
    rowacc_d = nc.dram_tensor("rowacc", [JC, FREE], f16, kind="ExternalOutput")

    n_gp = int(round(GROUPS * GPSIMD_ROWACC))

    with TileContext(nc) as tc:
        with (
            tc.tile_pool(name="const", bufs=1) as const_pool,
            tc.tile_pool(name="copies", bufs=4) as copy_pool,
            tc.tile_pool(name="psum", bufs=2, space="PSUM") as psum_pool,
        ):
            ab_sb = const_pool.tile([KSPLIT, I_PER_CORE + N], f16)
            nc.sync.dma_start(ab_sb[:], ab_aug[:])
            a_sb = ab_sb[:, :I_PER_CORE]
            b_sb = ab_sb[:, I_PER_CORE:]

            colmin_sb = const_pool.tile([JC, GROUPS], f32)

            racc = [
                const_pool.tile([JC, FREE], f16, name=f"racc{i}") for i in range(2)
            ]
            nc.gpsimd.memset(racc[0][:], 60000.0)

            for c in range(GROUPS):
                pt = psum_pool.tile([JC, FREE], f32, tag="pt")
                for s in range(FREE // MM_N):
                    nc.tensor.matmul(
                        pt[:, s * MM_N : (s + 1) * MM_N],
                        b_sb[:, c * JC : (c + 1) * JC],
                        a_sb[:, s * MM_N : (s + 1) * MM_N],
                        start=True,
                        stop=True,
                    )
                dcopy = copy_pool.tile([JC, FREE], f16, tag="dcopy")
                nc.vector._custom_dve(
                    minred,
                    out=dcopy[:],
                    in0=pt[:],
                    s0=3.0e38,
                    accum_out=colmin_sb[:, c : c + 1],
                )
                # Running min across groups; ping-pong buffers.
                eng = nc.gpsimd if c < n_gp else nc.vector
                eng.tensor_tensor(
                    racc[(c + 1) % 2][:],
                    racc[c % 2][:],
                    dcopy[:],
                    mybir.AluOpType.min,
                )

            nc.sync.dma_start(colmin_d[:], colmin_sb[:])
            nc.sync.dma_start(rowacc_d[:], racc[GROUPS % 2][:])

    nc.compile()
    return nc


def _augment(pts):
    """pts [N, 3] f32 -> (ahat15 [15, N], bhat15 [15, N]) fp16 hi/lo split.

    ahat = [|a|^2, 1, ax, ay, az]; bhat = [1, |b|^2, -2bx, -2by, -2bz]
    so ahat.bhat = ||a - b||^2.  fp16 split (per column vector v):
    v = v_hi + v_lo + O(2^-22 |v|).  The K=15 layouts
        ahat15 = [a_hi; a_lo; a_hi],  bhat15 = [b_hi; b_hi; b_lo]
    give a_hi.b_hi + a_lo.b_hi + a_hi.b_lo = a.b - a_lo.b_lo - eps.
    """
    pts = np.asarray(pts, dtype=np.float32)
    sq = np.sum(pts * pts, axis=1, dtype=np.float32)
    n = pts.shape[0]
    ahat = np.empty((K, n), dtype=np.float32)
    ahat[0] = sq
    ahat[1] = 1.0
    ahat[2:5] = pts.T
    bhat = np.empty((K, n), dtype=np.float32)
    bhat[0] = 1.0
    bhat[1] = sq
    bhat[2:5] = -2.0 * pts.T

    a_hi = ahat.astype(np.float16)
    a_lo = (ahat - a_hi.astype(np.float32)).astype(np.float16)
    b_hi = bhat.astype(np.float16)
    b_lo = (bhat - b_hi.astype(np.float32)).astype(np.float16)
    ahat15 = np.concatenate([a_hi, a_lo, a_hi], axis=0)
    bhat15 = np.concatenate([b_hi, b_hi, b_lo], axis=0)
    return ahat15, bhat15


def kernel(state_x, target):
    global LAST_RESULTS
    from concourse.bass_utils import run_bass_kernel_spmd

    state_x = np.asarray(state_x, dtype=np.float32)
    target = np.asarray(target, dtype=np.float32)

    if "nc" not in _CACHE:
        _CACHE["nc"] = _build_nc()
    nc = _CACHE["nc"]

    ahat, _ = _augment(state_x)   # streaming side: state_x
    _, bhat = _augment(target)    # stationary side: target

    in_maps = []
    for k in range(N_CORES):
        sl = slice(k * I_PER_CORE, (k + 1) * I_PER_CORE)
        ab = np.concatenate([ahat[:, sl], bhat], axis=1)
        in_maps.append({"ab_aug": np.ascontiguousarray(ab)})

    res = run_bass_kernel_spmd(nc, in_maps, core_ids=list(range(N_CORES)))
    LAST_RESULTS = res

    # dist2[i] = min_j d(i, j): partition-residue min of the row accumulator.
    dist2 = np.empty(N, dtype=np.float32)
    # dist1[j] = min_i d(i, j): combine per-core partials.
    dist1 = np.full(N, np.inf, dtype=np.float32)
    for k in range(N_CORES):
        out = res.results[k]
        racc = out["rowacc"].astype(np.float32)       # [128, 2048]
        dist2[k * I_PER_CORE : (k + 1) * I_PER_CORE] = racc.min(axis=0)
        colmin = out["colmin"]                        # [128, 128] [p, c]
        dist1 = np.minimum(dist1, colmin.T.reshape(N))

    dist1 = np.maximum(dist1, 0.0)
    dist2 = np.maximum(dist2, 0.0)
    loss = 0.5 * (np.mean(np.sqrt(dist1), dtype=np.float32)
                  + np.mean(np.sqrt(dist2), dtype=np.float32)) * 10.0
    return np.float32(loss)


# revision 19
# speedup vs baseline: 2.0624x; 1.0304x over previous
"""Chamfer loss kernel for Trainium2 (8 NeuronCores, SPMD).

Math: loss = 10 * 0.5 * (mean(sqrt(dist1)) + mean(sqrt(dist2)))
  dist1[j] = min_i ||target_j - state_x_i||^2   (over all state_x)
  dist2[i] = min_j ||state_x_i - target_j||^2   (over all target)

Device strategy (per core k of 8):
  - i (state_x) is sharded: core k owns rows [2048k, 2048(k+1)).
  - j (target) is replicated (stationary matmul operand).
  - K=5 augmented vectors:  bhat_j = [1, |b|^2, -2bx, -2by, -2bz],
    ahat_i = [|a|^2, 1, ax, ay, az]  so  bhat_j . ahat_i = d(i, j).
  - Per group c (128 target points): PE computes d tile [128 j, 2048 i]
    into PSUM (4 banks, 4x N=512 f32 matmuls).
  - DVE tensor_tensor_reduce: out = max(d, 0) -> fp16 SBUF copy,
    accum_out = min over i  -> exact f32 per-target partial min.
  - DVE tensor_tensor(min) folds the fp16 copy into a running [128, 2048]
    accumulator (the per-state_x min lands in partition-residue form).
  - Host: partition-residue min, cross-core combine, sqrt/mean epilogue.
"""

import os

import numpy as np

N = 16384
N_CORES = 8
I_PER_CORE = N // N_CORES  # 2048 streaming points per core
JC = 128                   # stationary chunk (output partitions per group)
GROUPS = N // JC           # 128 groups per core
FREE = I_PER_CORE          # 2048 free-dim elements per group
MM_N = 512                 # one PSUM bank of f32 output per matmul
K = 5                      # augmented coordinate count
# fp16 hi/lo split: d = a_hi.b_hi + a_lo.b_hi + a_hi.b_lo (error ~2^-21)
KSPLIT = 3 * K             # contraction dim of the fp16 matmul

# Matmul input dtype: "f16" (hi/lo split, ~2^-21 accurate) or "bf16"
# (hi/lo split, ~2^-15 accurate) — bf16 may stream 2x faster on the PE.
MM_DTYPE = os.environ.get("CHAMFER_MM_DTYPE", "f16")

_CACHE = {}

# Results of the last hardware run (BassKernelResults); test harness reads
# this for exec_time_ns when BASS_TRACE=1.
LAST_RESULTS = None


def _get_minred_op():
    """Register (once) a custom DVE op:
        out       = relu(in0)            (clamp + dtype-converting copy)
        accum_out = min(s0, min_k out[k])  (free-axis min reduce)
    One 1x DVE pass fuses the PSUM drain, the clamp, the fp16 copy and the
    free-axis min.  (The native TENSOR_TENSOR_REDUCE ISA opcode is rejected
    by this walrus build, hence the custom-table route.)
    """
    if "minred" in _CACHE:
        return _CACHE["minred"]

    import numpy as np
    from concourse import dve_ops
    from concourse.dve_spec import Spec, Src0, C0, lower, minn, relu, _has_src1
    from concourse.dve_uop import DveOpSpec

    def _ref(in0, in1, c0, c1, c2):
        b = np.maximum(
            np.nan_to_num(
                in0.astype(np.float32), nan=0.0, posinf=np.inf, neginf=-np.inf
            ),
            0.0,
        )
        acc = np.minimum(c0, b.reshape(b.shape[0], -1).min(axis=-1, keepdims=True))
        return b, acc

    spec = Spec(body=relu(Src0), accum=minn, accum_init=C0, reference=_ref)
    op = dve_ops.DveOp("CHAMFER_RELU_MINRED", spec, subdim=False, uops_sha={})
    dve_ops.OPS.append(op)
    dve_ops._SUB_OPCODE_FOR_NAME[op.name] = (
        max(dve_ops._SUB_OPCODE_FOR_NAME.values()) + 1
    )
    dve_ops.CUSTOM_DVE_SPECS[op.name] = op.spec
    for ver in ("v3", "v4"):
        s = DveOpSpec(
            name=op.name,
            opcode=dve_ops.get_dve_sub_opcode(op.name),
            uops=lower(spec, ver=ver),
            rd1_en=_has_src1(spec),
        )
        op.uops_sha[ver] = s.sha(ver)
    _CACHE["minred"] = op
    return op


def _build_nc():
    import concourse.mybir as mybir
    from concourse import bacc
    from concourse.tile import TileContext

    f32 = mybir.dt.float32
    f16 = mybir.dt.float16
    mmdt = f16 if MM_DTYPE == "f16" else mybir.dt.bfloat16
    Op = mybir.AluOpType

    nc = bacc.Bacc(
        "TRN2",
        target_bir_lowering=False,
        debug=False,
        enable_asserts=True,
        num_devices=N_CORES,
    )

    # One input tensor (single DMA → single wait sem on the first matmul):
    # [:, :I_PER_CORE] = streaming ahat slice, [:, I_PER_CORE:] = full bhat.
    ab_aug = nc.dram_tensor(
        "ab_aug", [KSPLIT, I_PER_CORE + N], mmdt, kind="ExternalInput"
    )
    colmin_d = nc.dram_tensor("colmin", [JC, GROUPS], f32, kind="ExternalOutput")
    rowacc_d = nc.dram_tensor("rowacc", [JC, FREE], f16, kind="ExternalOutput")

    QB = 4  # groups per DVE batch (amortizes DVE op init/tail overhead)
    H1, H2 = FREE // 2, FREE // 4  # 1024, 512

    with TileContext(nc) as tc:
        with (
            tc.tile_pool(name="const", bufs=1) as const_pool,
            tc.tile_pool(name="copies", bufs=2) as copy_pool,
            tc.tile_pool(name="tree", bufs=2) as tree_pool,
            tc.tile_pool(name="psum", bufs=2, space="PSUM") as psum_pool,
        ):
            # Stationary/moving operands replicated at partition bases
            # {0,32,64,96} so 4 matmuls can run concurrently in distinct
            # PE row groups (tile_position packing; K=15 fits in 32 rows).
            a_rep = const_pool.tile([128, I_PER_CORE], mmdt)
            b_rep = const_pool.tile([128, N], mmdt)
            for g in range(4):
                nc.sync.dma_start(
                    a_rep[32 * g : 32 * g + KSPLIT, :], ab_aug[:, :I_PER_CORE]
                )
                nc.sync.dma_start(
                    b_rep[32 * g : 32 * g + KSPLIT, :], ab_aug[:, I_PER_CORE:]
                )

            colmin_sb = const_pool.tile([JC, GROUPS], f32)

            racc = [
                const_pool.tile([JC, FREE], f16, name=f"racc{i}") for i in range(2)
            ]

            for q in range(GROUPS // QB):
                dcq = copy_pool.tile([JC, QB, FREE], f16, tag="dcq")
                for g in range(QB):
                    c = q * QB + g
                    pt = psum_pool.tile([JC, FREE], f32, tag="pt")
                    for s in range(FREE // MM_N):
                        nc.tensor.matmul(
                            pt[:, s * MM_N : (s + 1) * MM_N],
                            b_rep[32 * s : 32 * s + KSPLIT, c * JC : (c + 1) * JC],
                            a_rep[32 * s : 32 * s + KSPLIT, s * MM_N : (s + 1) * MM_N],
                            start=True,
                            stop=True,
                            tile_position=(32 * s, 0),
                        )
                    # ACT drains PSUM: clamp to >=0 + fp16 downcast.
                    nc.scalar.activation(
                        dcq[:, g, :], pt[:], mybir.ActivationFunctionType.Relu
                    )
                    # DVE row-accumulate (fp16 2x); ping-pong buffers.
                    if c == 0:
                        nc.vector.tensor_copy(racc[1][:], dcq[:, 0, :])
                    else:
                        nc.vector.tensor_tensor(
                            racc[(c + 1) % 2][:], racc[c % 2][:], dcq[:, g, :],
                            Op.min,
                        )
                # DVE column-min: quad-batched 2-level fp16 2x tree + reduce.
                t1q = tree_pool.tile([JC, QB, H1], f16, tag="t1q")
                nc.vector.tensor_tensor(
                    t1q[:], dcq[:, :, :H1], dcq[:, :, H1:], Op.min
                )
                t2q = tree_pool.tile([JC, QB, H2], f16, tag="t2q")
                nc.vector.tensor_tensor(
                    t2q[:], t1q[:, :, :H2], t1q[:, :, H2:], Op.min
                )
                nc.vector.tensor_reduce(
                    out=colmin_sb[:, q * QB : (q + 1) * QB],
                    in_=t2q[:],
                    axis=mybir.AxisListType.X,
                    op=Op.min,
                )

            nc.sync.dma_start(colmin_d[:], colmin_sb[:])
            nc.sync.dma_start(rowacc_d[:], racc[GROUPS % 2][:])

    nc.compile()
    return nc


def _augment(pts):
    """pts [N, 3] f32 -> (ahat15 [15, N], bhat15 [15, N]) fp16 hi/lo split.

    ahat = [|a|^2, 1, ax, ay, az]; bhat = [1, |b|^2, -2bx, -2by, -2bz]
    so ahat.bhat = ||a - b||^2.  fp16 split (per column vector v):
    v = v_hi + v_lo + O(2^-22 |v|).  The K=15 layouts
        ahat15 = [a_hi; a_lo; a_hi],  bhat15 = [b_hi; b_hi; b_lo]
    give a_hi.b_hi + a_lo.b_hi + a_hi.b_lo = a.b - a_lo.b_lo - eps.
    """
    pts = np.asarray(pts, dtype=np.float32)
    sq = np.sum(pts * pts, axis=1, dtype=np.float32)
    n = pts.shape[0]
    ahat = np.empty((K, n), dtype=np.float32)
    ahat[0] = sq
    ahat[1] = 1.0
    ahat[2:5] = pts.T
    bhat = np.empty((K, n), dtype=np.float32)
    bhat[0] = 1.0
    bhat[1] = sq
    bhat[2:5] = -2.0 * pts.T

    if MM_DTYPE == "f16":
        dt = np.float16
    else:
        import ml_dtypes

        dt = ml_dtypes.bfloat16
    a_hi = ahat.astype(dt)
    a_lo = (ahat - a_hi.astype(np.float32)).astype(dt)
    b_hi = bhat.astype(dt)
    b_lo = (bhat - b_hi.astype(np.float32)).astype(dt)
    ahat15 = np.concatenate([a_hi, a_lo, a_hi], axis=0)
    bhat15 = np.concatenate([b_hi, b_hi, b_lo], axis=0)
    return ahat15, bhat15


def kernel(state_x, target):
    global LAST_RESULTS
    from concourse.bass_utils import run_bass_kernel_spmd

    state_x = np.asarray(state_x, dtype=np.float32)
    target = np.asarray(target, dtype=np.float32)

    if "nc" not in _CACHE:
        _CACHE["nc"] = _build_nc()
    nc = _CACHE["nc"]

    ahat, _ = _augment(state_x)   # streaming side: state_x
    _, bhat = _augment(target)    # stationary side: target

    in_maps = []
    for k in range(N_CORES):
        sl = slice(k * I_PER_CORE, (k + 1) * I_PER_CORE)
        ab = np.concatenate([ahat[:, sl], bhat], axis=1)
        in_maps.append({"ab_aug": np.ascontiguousarray(ab)})

    res = run_bass_kernel_spmd(nc, in_maps, core_ids=list(range(N_CORES)))
    LAST_RESULTS = res

    # dist2[i] = min_j d(i, j): partition-residue min of the row accumulator.
    dist2 = np.empty(N, dtype=np.float32)
    # dist1[j] = min_i d(i, j): combine per-core partials.
    dist1 = np.full(N, np.inf, dtype=np.float32)
    for k in range(N_CORES):
        out = res.results[k]
        racc = out["rowacc"].astype(np.float32)       # [128, 2048]
        dist2[k * I_PER_CORE : (k + 1) * I_PER_CORE] = racc.min(axis=0)
        colmin = out["colmin"]                        # [128, 128] [p, c]
        dist1 = np.minimum(dist1, colmin.T.reshape(N))

    dist1 = np.maximum(dist1, 0.0)
    dist2 = np.maximum(dist2, 0.0)
    loss = 0.5 * (np.mean(np.sqrt(dist1), dtype=np.float32)
                  + np.mean(np.sqrt(dist2), dtype=np.float32)) * 10.0
    return np.float32(loss)


# revision 22
# speedup vs baseline: 2.5536x; 1.2382x over previous
"""Chamfer loss kernel for Trainium2 (8 NeuronCores, SPMD).

Math: loss = 10 * 0.5 * (mean(sqrt(dist1)) + mean(sqrt(dist2)))
  dist1[j] = min_i ||target_j - state_x_i||^2   (over all state_x)
  dist2[i] = min_j ||state_x_i - target_j||^2   (over all target)

Device strategy (per core k of 8):
  - i (state_x) is sharded: core k owns rows [2048k, 2048(k+1)).
  - j (target) is replicated (stationary matmul operand).
  - K=5 augmented vectors:  bhat_j = [1, |b|^2, -2bx, -2by, -2bz],
    ahat_i = [|a|^2, 1, ax, ay, az]  so  bhat_j . ahat_i = d(i, j).
  - Per group c (128 target points): PE computes d tile [128 j, 2048 i]
    into PSUM (4 banks, 4x N=512 f32 matmuls).
  - DVE tensor_tensor_reduce: out = max(d, 0) -> fp16 SBUF copy,
    accum_out = min over i  -> exact f32 per-target partial min.
  - DVE tensor_tensor(min) folds the fp16 copy into a running [128, 2048]
    accumulator (the per-state_x min lands in partition-residue form).
  - Host: partition-residue min, cross-core combine, sqrt/mean epilogue.
"""

import os

import numpy as np

N = 16384
N_CORES = 8
I_PER_CORE = N // N_CORES  # 2048 streaming points per core
JC = 128                   # stationary chunk (output partitions per group)
GROUPS = N // JC           # 128 groups per core
FREE = I_PER_CORE          # 2048 free-dim elements per group
MM_N = 512                 # one PSUM bank of f32 output per matmul
K = 5                      # augmented coordinate count
# fp16 hi/lo split: d = a_hi.b_hi + a_lo.b_hi + a_hi.b_lo (error ~2^-21)
KSPLIT = 3 * K             # contraction dim of the fp16 matmul

# Matmul input dtype: "f16" (hi/lo split, ~2^-21 accurate) or "bf16"
# (hi/lo split, ~2^-15 accurate) — bf16 may stream 2x faster on the PE.
MM_DTYPE = os.environ.get("CHAMFER_MM_DTYPE", "f16")
# PE row-group packing (4 concurrent matmuls via tile_position).
PACK = os.environ.get("CHAMFER_PACK", "0") == "1"

_CACHE = {}

# Results of the last hardware run (BassKernelResults); test harness reads
# this for exec_time_ns when BASS_TRACE=1.
LAST_RESULTS = None


def _get_minred_op():
    """Register (once) a custom DVE op:
        out       = relu(in0)            (clamp + dtype-converting copy)
        accum_out = min(s0, min_k out[k])  (free-axis min reduce)
    One 1x DVE pass fuses the PSUM drain, the clamp, the fp16 copy and the
    free-axis min.  (The native TENSOR_TENSOR_REDUCE ISA opcode is rejected
    by this walrus build, hence the custom-table route.)
    """
    if "minred" in _CACHE:
        return _CACHE["minred"]

    import numpy as np
    from concourse import dve_ops
    from concourse.dve_spec import Spec, Src0, C0, lower, minn, relu, _has_src1
    from concourse.dve_uop import DveOpSpec

    def _ref(in0, in1, c0, c1, c2):
        b = np.maximum(
            np.nan_to_num(
                in0.astype(np.float32), nan=0.0, posinf=np.inf, neginf=-np.inf
            ),
            0.0,
        )
        acc = np.minimum(c0, b.reshape(b.shape[0], -1).min(axis=-1, keepdims=True))
        return b, acc

    spec = Spec(body=relu(Src0), accum=minn, accum_init=C0, reference=_ref)
    op = dve_ops.DveOp("CHAMFER_RELU_MINRED", spec, subdim=False, uops_sha={})
    dve_ops.OPS.append(op)
    dve_ops._SUB_OPCODE_FOR_NAME[op.name] = (
        max(dve_ops._SUB_OPCODE_FOR_NAME.values()) + 1
    )
    dve_ops.CUSTOM_DVE_SPECS[op.name] = op.spec
    for ver in ("v3", "v4"):
        s = DveOpSpec(
            name=op.name,
            opcode=dve_ops.get_dve_sub_opcode(op.name),
            uops=lower(spec, ver=ver),
            rd1_en=_has_src1(spec),
        )
        op.uops_sha[ver] = s.sha(ver)
    _CACHE["minred"] = op
    return op


def _build_nc():
    import concourse.mybir as mybir
    from concourse import bacc
    from concourse.tile import TileContext

    f32 = mybir.dt.float32
    f16 = mybir.dt.float16
    mmdt = f16 if MM_DTYPE == "f16" else mybir.dt.bfloat16
    Op = mybir.AluOpType

    nc = bacc.Bacc(
        "TRN2",
        target_bir_lowering=False,
        debug=False,
        enable_asserts=True,
        num_devices=N_CORES,
    )

    # One input tensor (single DMA → single wait sem on the first matmul):
    # [:, :I_PER_CORE] = streaming ahat slice, [:, I_PER_CORE:] = full bhat.
    ab_aug = nc.dram_tensor(
        "ab_aug", [KSPLIT, I_PER_CORE + N], mmdt, kind="ExternalInput"
    )
    colmin_d = nc.dram_tensor("colmin", [JC, GROUPS], f32, kind="ExternalOutput")
    rowacc_d = nc.dram_tensor("rowacc", [JC, FREE], f16, kind="ExternalOutput")

    QB = 4  # groups per DVE batch (amortizes DVE op init/tail overhead)
    H1, H2 = FREE // 2, FREE // 4  # 1024, 512

    with TileContext(nc) as tc:
        with (
            tc.tile_pool(name="const", bufs=1) as const_pool,
            tc.tile_pool(name="copies", bufs=2) as copy_pool,
            tc.tile_pool(name="tree", bufs=2) as tree_pool,
            tc.tile_pool(name="psum", bufs=2, space="PSUM") as psum_pool,
        ):
            # Stationary/moving operands replicated at partition bases
            # {0,32,64,96} so 4 matmuls can run concurrently in distinct
            # PE row groups (tile_position packing; K=15 fits in 32 rows).
            n_rep = 4 if PACK else 1
            a_rep = const_pool.tile([32 * (n_rep - 1) + KSPLIT, I_PER_CORE], mmdt)
            b_rep = const_pool.tile([32 * (n_rep - 1) + KSPLIT, N], mmdt)
            for g in range(n_rep):
                nc.sync.dma_start(
                    a_rep[32 * g : 32 * g + KSPLIT, :], ab_aug[:, :I_PER_CORE]
                )
                nc.sync.dma_start(
                    b_rep[32 * g : 32 * g + KSPLIT, :], ab_aug[:, I_PER_CORE:]
                )

            colmin_sb = const_pool.tile([JC, GROUPS], f32)

            racc = [
                const_pool.tile([JC, FREE], f16, name=f"racc{i}") for i in range(2)
            ]

            for q in range(GROUPS // QB):
                dcq = copy_pool.tile([JC, QB, FREE], f16, tag="dcq")
                for g in range(QB):
                    c = q * QB + g
                    pt = psum_pool.tile([JC, FREE], f32, tag="pt")
                    for s in range(FREE // MM_N):
                        base = 32 * s if PACK else 0
                        kwargs = {"tile_position": (base, 0)} if PACK else {}
                        nc.tensor.matmul(
                            pt[:, s * MM_N : (s + 1) * MM_N],
                            b_rep[base : base + KSPLIT, c * JC : (c + 1) * JC],
                            a_rep[base : base + KSPLIT, s * MM_N : (s + 1) * MM_N],
                            start=True,
                            stop=True,
                            **kwargs,
                        )
                    # ACT drains PSUM: clamp to >=0 + fp16 downcast.
                    nc.scalar.activation(
                        dcq[:, g, :], pt[:], mybir.ActivationFunctionType.Relu
                    )
                    # DVE row-accumulate (fp16 2x); ping-pong buffers.
                    if c == 0:
                        nc.vector.tensor_copy(racc[1][:], dcq[:, 0, :])
                    else:
                        nc.vector.tensor_tensor(
                            racc[(c + 1) % 2][:], racc[c % 2][:], dcq[:, g, :],
                            Op.min,
                        )
                # DVE column-min: quad-batched 2-level fp16 2x tree + reduce.
                t1q = tree_pool.tile([JC, QB, H1], f16, tag="t1q")
                nc.vector.tensor_tensor(
                    t1q[:], dcq[:, :, :H1], dcq[:, :, H1:], Op.min
                )
                t2q = tree_pool.tile([JC, QB, H2], f16, tag="t2q")
                nc.vector.tensor_tensor(
                    t2q[:], t1q[:, :, :H2], t1q[:, :, H2:], Op.min
                )
                nc.vector.tensor_reduce(
                    out=colmin_sb[:, q * QB : (q + 1) * QB],
                    in_=t2q[:],
                    axis=mybir.AxisListType.X,
                    op=Op.min,
                )

            nc.sync.dma_start(colmin_d[:], colmin_sb[:])
            nc.sync.dma_start(rowacc_d[:], racc[GROUPS % 2][:])

    nc.compile()
    return nc


def _augment(pts):
    """pts [N, 3] f32 -> (ahat15 [15, N], bhat15 [15, N]) fp16 hi/lo split.

    ahat = [|a|^2, 1, ax, ay, az]; bhat = [1, |b|^2, -2bx, -2by, -2bz]
    so ahat.bhat = ||a - b||^2.  fp16 split (per column vector v):
    v = v_hi + v_lo + O(2^-22 |v|).  The K=15 layouts
        ahat15 = [a_hi; a_lo; a_hi],  bhat15 = [b_hi; b_hi; b_lo]
    give a_hi.b_hi + a_lo.b_hi + a_hi.b_lo = a.b - a_lo.b_lo - eps.
    """
    pts = np.asarray(pts, dtype=np.float32)
    sq = np.sum(pts * pts, axis=1, dtype=np.float32)
    n = pts.shape[0]
    ahat = np.empty((K, n), dtype=np.float32)
    ahat[0] = sq
    ahat[1] = 1.0
    ahat[2:5] = pts.T
    bhat = np.empty((K, n), dtype=np.float32)
    bhat[0] = 1.0
    bhat[1] = sq
    bhat[2:5] = -2.0 * pts.T

    if MM_DTYPE == "f16":
        dt = np.float16
    else:
        import ml_dtypes

        dt = ml_dtypes.bfloat16
    a_hi = ahat.astype(dt)
    a_lo = (ahat - a_hi.astype(np.float32)).astype(dt)
    b_hi = bhat.astype(dt)
    b_lo = (bhat - b_hi.astype(np.float32)).astype(dt)
    ahat15 = np.concatenate([a_hi, a_lo, a_hi], axis=0)
    bhat15 = np.concatenate([b_hi, b_hi, b_lo], axis=0)
    return ahat15, bhat15


def kernel(state_x, target):
    global LAST_RESULTS
    from concourse.bass_utils import run_bass_kernel_spmd

    state_x = np.asarray(state_x, dtype=np.float32)
    target = np.asarray(target, dtype=np.float32)

    if "nc" not in _CACHE:
        _CACHE["nc"] = _build_nc()
    nc = _CACHE["nc"]

    ahat, _ = _augment(state_x)   # streaming side: state_x
    _, bhat = _augment(target)    # stationary side: target

    in_maps = []
    for k in range(N_CORES):
        sl = slice(k * I_PER_CORE, (k + 1) * I_PER_CORE)
        ab = np.concatenate([ahat[:, sl], bhat], axis=1)
        in_maps.append({"ab_aug": np.ascontiguousarray(ab)})

    res = run_bass_kernel_spmd(nc, in_maps, core_ids=list(range(N_CORES)))
    LAST_RESULTS = res

    # dist2[i] = min_j d(i, j): partition-residue min of the row accumulator.
    dist2 = np.empty(N, dtype=np.float32)
    # dist1[j] = min_i d(i, j): combine per-core partials.
    dist1 = np.full(N, np.inf, dtype=np.float32)
    for k in range(N_CORES):
        out = res.results[k]
        racc = out["rowacc"].astype(np.float32)       # [128, 2048]
        dist2[k * I_PER_CORE : (k + 1) * I_PER_CORE] = racc.min(axis=0)
        colmin = out["colmin"]                        # [128, 128] [p, c]
        dist1 = np.minimum(dist1, colmin.T.reshape(N))

    dist1 = np.maximum(dist1, 0.0)
    dist2 = np.maximum(dist2, 0.0)
    loss = 0.5 * (np.mean(np.sqrt(dist1), dtype=np.float32)
                  + np.mean(np.sqrt(dist2), dtype=np.float32)) * 10.0
    return np.float32(loss)


# revision 24
# speedup vs baseline: 2.7729x; 1.0859x over previous
"""Chamfer loss kernel for Trainium2 (8 NeuronCores, SPMD).

Math: loss = 10 * 0.5 * (mean(sqrt(dist1)) + mean(sqrt(dist2)))
  dist1[j] = min_i ||target_j - state_x_i||^2   (over all state_x)
  dist2[i] = min_j ||state_x_i - target_j||^2   (over all target)

Device strategy (per core k of 8):
  - i (state_x) is sharded: core k owns rows [2048k, 2048(k+1)).
  - j (target) is replicated (stationary matmul operand).
  - K=5 augmented vectors:  bhat_j = [1, |b|^2, -2bx, -2by, -2bz],
    ahat_i = [|a|^2, 1, ax, ay, az]  so  bhat_j . ahat_i = d(i, j).
  - Per group c (128 target points): PE computes d tile [128 j, 2048 i]
    into PSUM (4 banks, 4x N=512 f32 matmuls).
  - DVE tensor_tensor_reduce: out = max(d, 0) -> fp16 SBUF copy,
    accum_out = min over i  -> exact f32 per-target partial min.
  - DVE tensor_tensor(min) folds the fp16 copy into a running [128, 2048]
    accumulator (the per-state_x min lands in partition-residue form).
  - Host: partition-residue min, cross-core combine, sqrt/mean epilogue.
"""

import os

import numpy as np

N = 16384
N_CORES = 8
I_PER_CORE = N // N_CORES  # 2048 streaming points per core
JC = 128                   # stationary chunk (output partitions per group)
GROUPS = N // JC           # 128 groups per core
FREE = I_PER_CORE          # 2048 free-dim elements per group
MM_N = 512                 # one PSUM bank of f32 output per matmul
K = 5                      # augmented coordinate count
# fp16 hi/lo split: d = a_hi.b_hi + a_lo.b_hi + a_hi.b_lo (error ~2^-21)
KSPLIT = 3 * K             # contraction dim of the fp16 matmul

# Matmul input dtype: "f16" (hi/lo split, ~2^-21 accurate) or "bf16"
# (hi/lo split, ~2^-15 accurate) — bf16 may stream 2x faster on the PE.
MM_DTYPE = os.environ.get("CHAMFER_MM_DTYPE", "f16")
# PE row-group packing (4 concurrent matmuls via tile_position).
PACK = os.environ.get("CHAMFER_PACK", "0") == "1"

_CACHE = {}

# Results of the last hardware run (BassKernelResults); test harness reads
# this for exec_time_ns when BASS_TRACE=1.
LAST_RESULTS = None


def _get_minred_op():
    """Register (once) a custom DVE op:
        out       = relu(in0)            (clamp + dtype-converting copy)
        accum_out = min(s0, min_k out[k])  (free-axis min reduce)
    One 1x DVE pass fuses the PSUM drain, the clamp, the fp16 copy and the
    free-axis min.  (The native TENSOR_TENSOR_REDUCE ISA opcode is rejected
    by this walrus build, hence the custom-table route.)
    """
    if "minred" in _CACHE:
        return _CACHE["minred"]

    import numpy as np
    from concourse import dve_ops
    from concourse.dve_spec import Spec, Src0, C0, lower, minn, relu, _has_src1
    from concourse.dve_uop import DveOpSpec

    def _ref(in0, in1, c0, c1, c2):
        b = np.maximum(
            np.nan_to_num(
                in0.astype(np.float32), nan=0.0, posinf=np.inf, neginf=-np.inf
            ),
            0.0,
        )
        acc = np.minimum(c0, b.reshape(b.shape[0], -1).min(axis=-1, keepdims=True))
        return b, acc

    spec = Spec(body=relu(Src0), accum=minn, accum_init=C0, reference=_ref)
    op = dve_ops.DveOp("CHAMFER_RELU_MINRED", spec, subdim=False, uops_sha={})
    dve_ops.OPS.append(op)
    dve_ops._SUB_OPCODE_FOR_NAME[op.name] = (
        max(dve_ops._SUB_OPCODE_FOR_NAME.values()) + 1
    )
    dve_ops.CUSTOM_DVE_SPECS[op.name] = op.spec
    for ver in ("v3", "v4"):
        s = DveOpSpec(
            name=op.name,
            opcode=dve_ops.get_dve_sub_opcode(op.name),
            uops=lower(spec, ver=ver),
            rd1_en=_has_src1(spec),
        )
        op.uops_sha[ver] = s.sha(ver)
    _CACHE["minred"] = op
    return op


def _build_nc():
    import concourse.mybir as mybir
    from concourse import bacc
    from concourse.tile import TileContext

    f32 = mybir.dt.float32
    f16 = mybir.dt.float16
    mmdt = f16 if MM_DTYPE == "f16" else mybir.dt.bfloat16
    Op = mybir.AluOpType

    nc = bacc.Bacc(
        "TRN2",
        target_bir_lowering=False,
        debug=False,
        enable_asserts=True,
        num_devices=N_CORES,
    )

    # One input tensor (single DMA → single wait sem on the first matmul):
    # [:, :I_PER_CORE] = streaming ahat slice, [:, I_PER_CORE:] = full bhat.
    ab_aug = nc.dram_tensor(
        "ab_aug", [KSPLIT, I_PER_CORE + N], mmdt, kind="ExternalInput"
    )
    colmin_d = nc.dram_tensor("colmin", [JC, GROUPS], f32, kind="ExternalOutput")
    rowacc_d = nc.dram_tensor("rowacc", [JC, FREE], f16, kind="ExternalOutput")

    QB = 8  # groups per DVE batch (amortizes DVE op init/tail overhead)

    with TileContext(nc) as tc:
        with (
            tc.tile_pool(name="const", bufs=1) as const_pool,
            tc.tile_pool(name="copies", bufs=2) as copy_pool,
            tc.tile_pool(name="tree", bufs=1) as tree_pool,
            tc.tile_pool(name="psum", bufs=2, space="PSUM") as psum_pool,
        ):
            # Stationary/moving operands replicated at partition bases
            # {0,32,64,96} so 4 matmuls can run concurrently in distinct
            # PE row groups (tile_position packing; K=15 fits in 32 rows).
            n_rep = 4 if PACK else 1
            a_rep = const_pool.tile([32 * (n_rep - 1) + KSPLIT, I_PER_CORE], mmdt)
            b_rep = const_pool.tile([32 * (n_rep - 1) + KSPLIT, N], mmdt)
            for g in range(n_rep):
                nc.sync.dma_start(
                    a_rep[32 * g : 32 * g + KSPLIT, :], ab_aug[:, :I_PER_CORE]
                )
                nc.sync.dma_start(
                    b_rep[32 * g : 32 * g + KSPLIT, :], ab_aug[:, I_PER_CORE:]
                )

            colmin_sb = const_pool.tile([JC, GROUPS], f32)

            racc = [
                const_pool.tile([JC, FREE], f16, name=f"racc{i}") for i in range(2)
            ]

            n_oct = GROUPS // QB  # 16
            for q in range(n_oct):
                dcq = copy_pool.tile([JC, QB, FREE], f16, tag="dcq")
                for g in range(QB):
                    c = q * QB + g
                    pt = psum_pool.tile([JC, FREE], f32, tag="pt")
                    for s in range(FREE // MM_N):
                        nc.tensor.matmul(
                            pt[:, s * MM_N : (s + 1) * MM_N],
                            b_rep[:KSPLIT, c * JC : (c + 1) * JC],
                            a_rep[:KSPLIT, s * MM_N : (s + 1) * MM_N],
                            start=True,
                            stop=True,
                        )
                    # ACT drains PSUM: clamp to >=0 + fp16 downcast.
                    nc.scalar.activation(
                        dcq[:, g, :], pt[:], mybir.ActivationFunctionType.Relu
                    )
                # DVE row-accumulate: fold the 8 group slices pairwise
                # (all fp16 2x), then one chain update into racc.
                r1 = tree_pool.tile([JC, QB // 2, FREE], f16, tag="r1")
                nc.vector.tensor_tensor(
                    r1[:], dcq[:, 0 : QB // 2, :], dcq[:, QB // 2 :, :], Op.min
                )
                r2 = tree_pool.tile([JC, QB // 4, FREE], f16, tag="r2")
                nc.vector.tensor_tensor(
                    r2[:], r1[:, 0 : QB // 4, :], r1[:, QB // 4 :, :], Op.min
                )
                r3 = tree_pool.tile([JC, FREE], f16, tag="r3")
                nc.vector.tensor_tensor(r3[:], r2[:, 0, :], r2[:, 1, :], Op.min)
                if q == 0:
                    nc.vector.tensor_copy(racc[1][:], r3[:])
                else:
                    nc.vector.tensor_tensor(
                        racc[(q + 1) % 2][:], racc[q % 2][:], r3[:], Op.min
                    )
                # DVE column-min: batched fp16 2x tree to width 64 + reduce.
                w = FREE // 2
                src = dcq[:]
                for lvl in range(5):
                    t = tree_pool.tile([JC, QB, w], f16, tag=f"t{lvl}")
                    nc.vector.tensor_tensor(
                        t[:], src[:, :, :w], src[:, :, w:], Op.min
                    )
                    src = t[:]
                    w //= 2
                nc.vector.tensor_reduce(
                    out=colmin_sb[:, q * QB : (q + 1) * QB],
                    in_=src,
                    axis=mybir.AxisListType.X,
                    op=Op.min,
                )

            nc.sync.dma_start(colmin_d[:], colmin_sb[:])
            nc.sync.dma_start(rowacc_d[:], racc[n_oct % 2][:])

    nc.compile()
    return nc


def _augment(pts):
    """pts [N, 3] f32 -> (ahat15 [15, N], bhat15 [15, N]) fp16 hi/lo split.

    ahat = [|a|^2, 1, ax, ay, az]; bhat = [1, |b|^2, -2bx, -2by, -2bz]
    so ahat.bhat = ||a - b||^2.  fp16 split (per column vector v):
    v = v_hi + v_lo + O(2^-22 |v|).  The K=15 layouts
        ahat15 = [a_hi; a_lo; a_hi],  bhat15 = [b_hi; b_hi; b_lo]
    give a_hi.b_hi + a_lo.b_hi + a_hi.b_lo = a.b - a_lo.b_lo - eps.
    """
    pts = np.asarray(pts, dtype=np.float32)
    sq = np.sum(pts * pts, axis=1, dtype=np.float32)
    n = pts.shape[0]
    ahat = np.empty((K, n), dtype=np.float32)
    ahat[0] = sq
    ahat[1] = 1.0
    ahat[2:5] = pts.T
    bhat = np.empty((K, n), dtype=np.float32)
    bhat[0] = 1.0
    bhat[1] = sq
    bhat[2:5] = -2.0 * pts.T

    if MM_DTYPE == "f16":
        dt = np.float16
    else:
        import ml_dtypes

        dt = ml_dtypes.bfloat16
    a_hi = ahat.astype(dt)
    a_lo = (ahat - a_hi.astype(np.float32)).astype(dt)
    b_hi = bhat.astype(dt)
    b_lo = (bhat - b_hi.astype(np.float32)).astype(dt)
    ahat15 = np.concatenate([a_hi, a_lo, a_hi], axis=0)
    bhat15 = np.concatenate([b_hi, b_hi, b_lo], axis=0)
    return ahat15, bhat15


def kernel(state_x, target):
    global LAST_RESULTS
    from concourse.bass_utils import run_bass_kernel_spmd

    state_x = np.asarray(state_x, dtype=np.float32)
    target = np.asarray(target, dtype=np.float32)

    if "nc" not in _CACHE:
        _CACHE["nc"] = _build_nc()
    nc = _CACHE["nc"]

    ahat, _ = _augment(state_x)   # streaming side: state_x
    _, bhat = _augment(target)    # stationary side: target

    in_maps = []
    for k in range(N_CORES):
        sl = slice(k * I_PER_CORE, (k + 1) * I_PER_CORE)
        ab = np.concatenate([ahat[:, sl], bhat], axis=1)
        in_maps.append({"ab_aug": np.ascontiguousarray(ab)})

    res = run_bass_kernel_spmd(nc, in_maps, core_ids=list(range(N_CORES)))
    LAST_RESULTS = res

    # dist2[i] = min_j d(i, j): partition-residue min of the row accumulator.
    dist2 = np.empty(N, dtype=np.float32)
    # dist1[j] = min_i d(i, j): combine per-core partials.
    dist1 = np.full(N, np.inf, dtype=np.float32)
    for k in range(N_CORES):
        out = res.results[k]
        racc = out["rowacc"].astype(np.float32)       # [128, 2048]
        dist2[k * I_PER_CORE : (k + 1) * I_PER_CORE] = racc.min(axis=0)
        colmin = out["colmin"]                        # [128, 128] [p, c]
        dist1 = np.minimum(dist1, colmin.T.reshape(N))

    dist1 = np.maximum(dist1, 0.0)
    dist2 = np.maximum(dist2, 0.0)
    loss = 0.5 * (np.mean(np.sqrt(dist1), dtype=np.float32)
                  + np.mean(np.sqrt(dist2), dtype=np.float32)) * 10.0
    return np.float32(loss)


# revision 27
# speedup vs baseline: 2.8302x; 1.0207x over previous
"""Chamfer loss kernel for Trainium2 (8 NeuronCores, SPMD).

Math: loss = 10 * 0.5 * (mean(sqrt(dist1)) + mean(sqrt(dist2)))
  dist1[j] = min_i ||target_j - state_x_i||^2   (over all state_x)
  dist2[i] = min_j ||state_x_i - target_j||^2   (over all target)

Device strategy (per core k of 8):
  - i (state_x) is sharded: core k owns rows [2048k, 2048(k+1)).
  - j (target) is replicated (stationary matmul operand).
  - K=5 augmented vectors:  bhat_j = [1, |b|^2, -2bx, -2by, -2bz],
    ahat_i = [|a|^2, 1, ax, ay, az]  so  bhat_j . ahat_i = d(i, j).
  - Per group c (128 target points): PE computes d tile [128 j, 2048 i]
    into PSUM (4 banks, 4x N=512 f32 matmuls).
  - DVE tensor_tensor_reduce: out = max(d, 0) -> fp16 SBUF copy,
    accum_out = min over i  -> exact f32 per-target partial min.
  - DVE tensor_tensor(min) folds the fp16 copy into a running [128, 2048]
    accumulator (the per-state_x min lands in partition-residue form).
  - Host: partition-residue min, cross-core combine, sqrt/mean epilogue.
"""

import os

import numpy as np

N = 16384
N_CORES = 8
I_PER_CORE = N // N_CORES  # 2048 streaming points per core
JC = 128                   # stationary chunk (output partitions per group)
GROUPS = N // JC           # 128 groups per core
FREE = I_PER_CORE          # 2048 free-dim elements per group
MM_N = 512                 # one PSUM bank of f32 output per matmul
K = 5                      # augmented coordinate count
# fp16 hi/lo split: d = a_hi.b_hi + a_lo.b_hi + a_hi.b_lo (error ~2^-21)
KSPLIT = 3 * K             # contraction dim of the fp16 matmul

# Matmul input dtype: "f16" (hi/lo split, ~2^-21 accurate) or "bf16"
# (hi/lo split, ~2^-15 accurate) — bf16 may stream 2x faster on the PE.
MM_DTYPE = os.environ.get("CHAMFER_MM_DTYPE", "f16")
# PE row-group packing (4 concurrent matmuls via tile_position).
PACK = os.environ.get("CHAMFER_PACK", "0") == "1"

_CACHE = {}

# Results of the last hardware run (BassKernelResults); test harness reads
# this for exec_time_ns when BASS_TRACE=1.
LAST_RESULTS = None


def _get_minred_op():
    """Register (once) a custom DVE op:
        out       = relu(in0)            (clamp + dtype-converting copy)
        accum_out = min(s0, min_k out[k])  (free-axis min reduce)
    One 1x DVE pass fuses the PSUM drain, the clamp, the fp16 copy and the
    free-axis min.  (The native TENSOR_TENSOR_REDUCE ISA opcode is rejected
    by this walrus build, hence the custom-table route.)
    """
    if "minred" in _CACHE:
        return _CACHE["minred"]

    import numpy as np
    from concourse import dve_ops
    from concourse.dve_spec import Spec, Src0, C0, lower, minn, relu, _has_src1
    from concourse.dve_uop import DveOpSpec

    def _ref(in0, in1, c0, c1, c2):
        b = np.maximum(
            np.nan_to_num(
                in0.astype(np.float32), nan=0.0, posinf=np.inf, neginf=-np.inf
            ),
            0.0,
        )
        acc = np.minimum(c0, b.reshape(b.shape[0], -1).min(axis=-1, keepdims=True))
        return b, acc

    spec = Spec(body=relu(Src0), accum=minn, accum_init=C0, reference=_ref)
    op = dve_ops.DveOp("CHAMFER_RELU_MINRED", spec, subdim=False, uops_sha={})
    dve_ops.OPS.append(op)
    dve_ops._SUB_OPCODE_FOR_NAME[op.name] = (
        max(dve_ops._SUB_OPCODE_FOR_NAME.values()) + 1
    )
    dve_ops.CUSTOM_DVE_SPECS[op.name] = op.spec
    for ver in ("v3", "v4"):
        s = DveOpSpec(
            name=op.name,
            opcode=dve_ops.get_dve_sub_opcode(op.name),
            uops=lower(spec, ver=ver),
            rd1_en=_has_src1(spec),
        )
        op.uops_sha[ver] = s.sha(ver)
    _CACHE["minred"] = op
    return op


def _build_nc():
    import concourse.mybir as mybir
    from concourse import bacc
    from concourse.tile import TileContext

    f32 = mybir.dt.float32
    f16 = mybir.dt.float16
    mmdt = f16 if MM_DTYPE == "f16" else mybir.dt.bfloat16
    Op = mybir.AluOpType

    nc = bacc.Bacc(
        "TRN2",
        target_bir_lowering=False,
        debug=False,
        enable_asserts=True,
        num_devices=N_CORES,
    )

    # One input tensor (single DMA → single wait sem on the first matmul):
    # [:, :I_PER_CORE] = streaming ahat slice, [:, I_PER_CORE:] = full bhat.
    ab_aug = nc.dram_tensor(
        "ab_aug", [KSPLIT, I_PER_CORE + N], mmdt, kind="ExternalInput"
    )
    colmin_d = nc.dram_tensor("colmin", [JC, GROUPS], f32, kind="ExternalOutput")
    rowacc_d = nc.dram_tensor("rowacc", [JC, FREE], f16, kind="ExternalOutput")

    QB = 8  # groups per DVE batch (amortizes DVE op init/tail overhead)

    with TileContext(nc) as tc:
        with (
            tc.tile_pool(name="const", bufs=1) as const_pool,
            tc.tile_pool(name="copies", bufs=2) as copy_pool,
            tc.tile_pool(name="tree", bufs=1) as tree_pool,
            tc.tile_pool(name="psum", bufs=2, space="PSUM") as psum_pool,
        ):
            # Stationary/moving operands replicated at partition bases
            # {0,32,64,96} so 4 matmuls can run concurrently in distinct
            # PE row groups (tile_position packing; K=15 fits in 32 rows).
            a_rep = const_pool.tile([KSPLIT, I_PER_CORE], mmdt)
            b_rep = const_pool.tile([KSPLIT, N], mmdt)
            nc.sync.dma_start(a_rep[:], ab_aug[:, :I_PER_CORE])
            # Split the stationary-side DMA so group 0's matmuls only wait
            # for the first slice.
            nc.sync.dma_start(
                b_rep[:, : 8 * JC], ab_aug[:, I_PER_CORE : I_PER_CORE + 8 * JC]
            )
            nc.sync.dma_start(
                b_rep[:, 8 * JC :], ab_aug[:, I_PER_CORE + 8 * JC :]
            )

            colmin_sb = const_pool.tile([JC, GROUPS], f32)

            racc = [
                const_pool.tile([JC, FREE], f16, name=f"racc{i}") for i in range(2)
            ]

            # Ramp-up batch sizes: DVE work starts after one group instead
            # of a full batch of 8.
            batches = [1, 1, 2, 4] + [QB] * ((GROUPS - 8) // QB)
            assert sum(batches) == GROUPS
            off = 0  # first group of this batch
            for bi, nb in enumerate(batches):
                dcq = copy_pool.tile([JC, QB, FREE], f16, tag="dcq")
                for g in range(nb):
                    c = off + g
                    pt = psum_pool.tile([JC, FREE], f32, tag="pt")
                    for s in range(FREE // MM_N):
                        nc.tensor.matmul(
                            pt[:, s * MM_N : (s + 1) * MM_N],
                            b_rep[:, c * JC : (c + 1) * JC],
                            a_rep[:, s * MM_N : (s + 1) * MM_N],
                            start=True,
                            stop=True,
                        )
                    # ACT drains PSUM: clamp to >=0 + fp16 downcast.
                    nc.scalar.activation(
                        dcq[:, g, :], pt[:], mybir.ActivationFunctionType.Relu
                    )
                # DVE row-accumulate: fold the batch's group slices pairwise
                # (all fp16 2x), then one chain update into racc.
                src = dcq[:, :nb, :]
                m = nb
                while m > 1:
                    t = tree_pool.tile([JC, m // 2, FREE], f16, tag=f"r{m}")
                    nc.vector.tensor_tensor(
                        t[:], src[:, 0 : m // 2, :], src[:, m // 2 : m, :], Op.min
                    )
                    src = t[:]
                    m //= 2
                # src is [JC, 1, FREE] (or the dcq slice when nb == 1)
                if bi == 0:
                    nc.vector.tensor_copy(racc[1][:], dcq[:, 0, :])
                else:
                    nc.vector.tensor_tensor(
                        racc[(bi + 1) % 2][:],
                        racc[bi % 2][:],
                        src[:, 0, :],
                        Op.min,
                    )
                # DVE column-min: batched fp16 2x tree to width 64 + reduce.
                w = FREE // 2
                src = dcq[:, :nb, :]
                while w >= 64:
                    t = tree_pool.tile([JC, QB, w], f16, tag=f"t{w}")
                    nc.vector.tensor_tensor(
                        t[:, :nb, :], src[:, :, :w], src[:, :, w:], Op.min
                    )
                    src = t[:, :nb, :]
                    w //= 2
                nc.vector.tensor_reduce(
                    out=colmin_sb[:, off : off + nb],
                    in_=src,
                    axis=mybir.AxisListType.X,
                    op=Op.min,
                )
                off += nb

            nc.sync.dma_start(colmin_d[:], colmin_sb[:])
            nc.sync.dma_start(rowacc_d[:], racc[len(batches) % 2][:])

    nc.compile()
    return nc


def _augment(pts):
    """pts [N, 3] f32 -> (ahat15 [15, N], bhat15 [15, N]) fp16 hi/lo split.

    ahat = [|a|^2, 1, ax, ay, az]; bhat = [1, |b|^2, -2bx, -2by, -2bz]
    so ahat.bhat = ||a - b||^2.  fp16 split (per column vector v):
    v = v_hi + v_lo + O(2^-22 |v|).  The K=15 layouts
        ahat15 = [a_hi; a_lo; a_hi],  bhat15 = [b_hi; b_hi; b_lo]
    give a_hi.b_hi + a_lo.b_hi + a_hi.b_lo = a.b - a_lo.b_lo - eps.
    """
    pts = np.asarray(pts, dtype=np.float32)
    sq = np.sum(pts * pts, axis=1, dtype=np.float32)
    n = pts.shape[0]
    ahat = np.empty((K, n), dtype=np.float32)
    ahat[0] = sq
    ahat[1] = 1.0
    ahat[2:5] = pts.T
    bhat = np.empty((K, n), dtype=np.float32)
    bhat[0] = 1.0
    bhat[1] = sq
    bhat[2:5] = -2.0 * pts.T

    if MM_DTYPE == "f16":
        dt = np.float16
    else:
        import ml_dtypes

        dt = ml_dtypes.bfloat16
    a_hi = ahat.astype(dt)
    a_lo = (ahat - a_hi.astype(np.float32)).astype(dt)
    b_hi = bhat.astype(dt)
    b_lo = (bhat - b_hi.astype(np.float32)).astype(dt)
    ahat15 = np.concatenate([a_hi, a_lo, a_hi], axis=0)
    bhat15 = np.concatenate([b_hi, b_hi, b_lo], axis=0)
    return ahat15, bhat15


def kernel(state_x, target):
    global LAST_RESULTS
    from concourse.bass_utils import run_bass_kernel_spmd

    state_x = np.asarray(state_x, dtype=np.float32)
    target = np.asarray(target, dtype=np.float32)

    if "nc" not in _CACHE:
        _CACHE["nc"] = _build_nc()
    nc = _CACHE["nc"]

    ahat, _ = _augment(state_x)   # streaming side: state_x
    _, bhat = _augment(target)    # stationary side: target

    in_maps = []
    for k in range(N_CORES):
        sl = slice(k * I_PER_CORE, (k + 1) * I_PER_CORE)
        ab = np.concatenate([ahat[:, sl], bhat], axis=1)
        in_maps.append({"ab_aug": np.ascontiguousarray(ab)})

    res = run_bass_kernel_spmd(nc, in_maps, core_ids=list(range(N_CORES)))
    LAST_RESULTS = res

    # dist2[i] = min_j d(i, j): partition-residue min of the row accumulator.
    dist2 = np.empty(N, dtype=np.float32)
    # dist1[j] = min_i d(i, j): combine per-core partials.
    dist1 = np.full(N, np.inf, dtype=np.float32)
    for k in range(N_CORES):
        out = res.results[k]
        racc = out["rowacc"].astype(np.float32)       # [128, 2048]
        dist2[k * I_PER_CORE : (k + 1) * I_PER_CORE] = racc.min(axis=0)
        colmin = out["colmin"]                        # [128, 128] [p, c]
        dist1 = np.minimum(dist1, colmin.T.reshape(N))

    dist1 = np.maximum(dist1, 0.0)
    dist2 = np.maximum(dist2, 0.0)
    loss = 0.5 * (np.mean(np.sqrt(dist1), dtype=np.float32)
                  + np.mean(np.sqrt(dist2), dtype=np.float32)) * 10.0
    return np.float32(loss)


# revision 30
# speedup vs baseline: 2.8308x; 1.0002x over previous
"""Chamfer loss kernel for Trainium2 (8 NeuronCores, SPMD).

Math: loss = 10 * 0.5 * (mean(sqrt(dist1)) + mean(sqrt(dist2)))
  dist1[j] = min_i ||target_j - state_x_i||^2   (over all state_x)
  dist2[i] = min_j ||state_x_i - target_j||^2   (over all target)

Device strategy (per core k of 8):
  - i (state_x) is sharded: core k owns rows [2048k, 2048(k+1)).
  - j (target) is replicated (stationary matmul operand).
  - Augmented vectors  bhat_j = [1, |b|^2, -2bx, -2by, -2bz],
    ahat_i = [|a|^2, 1, ax, ay, az]  give  bhat_j . ahat_i = d(i, j);
    each side is fp16 hi/lo split (K = 15) so the PE computes the full
    squared-distance tile at near-f32 accuracy.
  - Per group c (128 target points): PE computes the d tile [128 j, 2048 i]
    into PSUM (4 banks, 4x N=512 matmuls).
  - ScalarE (ACT) drains PSUM: relu-clamp + fp16 downcast into SBUF
    (batches of 8 groups in one [128, 8, 2048] tile).
  - VectorE (DVE, the min monopoly) runs two fp16 2x-mode tensor_tensor
    min trees per batch: one folding the batch dim (-> running per-i
    accumulator racc, partition-residue form) and one folding the free
    dim to width 64 + a 1x tensor_reduce (-> exact per-j column mins).
  - Host: partition-residue min, cross-core combine, sqrt/mean epilogue.
"""

import os

import numpy as np

N = 16384
N_CORES = 8
I_PER_CORE = N // N_CORES  # 2048 streaming points per core
JC = 128                   # stationary chunk (output partitions per group)
GROUPS = N // JC           # 128 groups per core
FREE = I_PER_CORE          # 2048 free-dim elements per group
MM_N = 512                 # one PSUM bank of f32 output per matmul
K = 5                      # augmented coordinate count
# fp16 hi/lo split: d = a_hi.b_hi + a_lo.b_hi + a_hi.b_lo (error ~2^-21)
KSPLIT = 3 * K             # contraction dim of the fp16 matmul

# Matmul input dtype: "f16" (hi/lo split, ~2^-21 accurate) or "bf16"
# (hi/lo split, ~2^-15 accurate) — bf16 may stream 2x faster on the PE.
MM_DTYPE = os.environ.get("CHAMFER_MM_DTYPE", "f16")

_CACHE = {}

# Results of the last hardware run (BassKernelResults); test harness reads
# this for exec_time_ns when BASS_TRACE=1.
LAST_RESULTS = None


def _build_nc():
    import concourse.mybir as mybir
    from concourse import bacc
    from concourse.tile import TileContext

    f32 = mybir.dt.float32
    f16 = mybir.dt.float16
    mmdt = f16 if MM_DTYPE == "f16" else mybir.dt.bfloat16
    Op = mybir.AluOpType

    nc = bacc.Bacc(
        "TRN2",
        target_bir_lowering=False,
        debug=False,
        enable_asserts=True,
        num_devices=N_CORES,
    )

    # One input tensor (single DMA → single wait sem on the first matmul):
    # [:, :I_PER_CORE] = streaming ahat slice, [:, I_PER_CORE:] = full bhat.
    ab_aug = nc.dram_tensor(
        "ab_aug", [KSPLIT, I_PER_CORE + N], mmdt, kind="ExternalInput"
    )
    colmin_d = nc.dram_tensor("colmin", [JC, GROUPS], f32, kind="ExternalOutput")
    rowacc_d = nc.dram_tensor("rowacc", [JC, FREE], f16, kind="ExternalOutput")

    QB = 8  # groups per DVE batch (amortizes DVE op init/tail overhead)

    with TileContext(nc) as tc:
        with (
            tc.tile_pool(name="const", bufs=1) as const_pool,
            tc.tile_pool(name="copies", bufs=2) as copy_pool,
            tc.tile_pool(name="tree", bufs=1) as tree_pool,
            tc.tile_pool(name="psum", bufs=2, space="PSUM") as psum_pool,
        ):
            a_rep = const_pool.tile([KSPLIT, I_PER_CORE], mmdt)
            b_rep = const_pool.tile([KSPLIT, N], mmdt)
            nc.sync.dma_start(a_rep[:], ab_aug[:, :I_PER_CORE])
            # Split the stationary-side DMA so group 0's matmuls only wait
            # for the first slice.
            nc.sync.dma_start(
                b_rep[:, : 8 * JC], ab_aug[:, I_PER_CORE : I_PER_CORE + 8 * JC]
            )
            nc.sync.dma_start(
                b_rep[:, 8 * JC :], ab_aug[:, I_PER_CORE + 8 * JC :]
            )

            colmin_sb = const_pool.tile([JC, GROUPS], f32)

            racc = [
                const_pool.tile([JC, FREE], f16, name=f"racc{i}") for i in range(2)
            ]

            # Ramp-up batch sizes: DVE work starts after one group instead
            # of a full batch of 8.
            batches = [1, 1, 2, 4] + [QB] * ((GROUPS - 8) // QB)
            assert sum(batches) == GROUPS
            off = 0  # first group of this batch
            for bi, nb in enumerate(batches):
                dcq = copy_pool.tile([JC, QB, FREE], f16, tag="dcq")
                for g in range(nb):
                    c = off + g
                    pt = psum_pool.tile([JC, FREE], f32, tag="pt")
                    for s in range(FREE // MM_N):
                        nc.tensor.matmul(
                            pt[:, s * MM_N : (s + 1) * MM_N],
                            b_rep[:, c * JC : (c + 1) * JC],
                            a_rep[:, s * MM_N : (s + 1) * MM_N],
                            start=True,
                            stop=True,
                        )
                    # ACT drains PSUM: clamp to >=0 + fp16 downcast.
                    nc.scalar.activation(
                        dcq[:, g, :], pt[:], mybir.ActivationFunctionType.Relu
                    )
                # DVE row-accumulate: fold the batch's group slices pairwise
                # (all fp16 2x), then one chain update into racc.
                src = dcq[:, :nb, :]
                m = nb
                while m > 1:
                    t = tree_pool.tile([JC, m // 2, FREE], f16, tag=f"r{m}")
                    nc.vector.tensor_tensor(
                        t[:], src[:, 0 : m // 2, :], src[:, m // 2 : m, :], Op.min
                    )
                    src = t[:]
                    m //= 2
                # src is [JC, 1, FREE] (or the dcq slice when nb == 1)
                if bi == 0:
                    nc.vector.tensor_copy(racc[1][:], dcq[:, 0, :])
                else:
                    nc.vector.tensor_tensor(
                        racc[(bi + 1) % 2][:],
                        racc[bi % 2][:],
                        src[:, 0, :],
                        Op.min,
                    )
                # DVE column-min: batched fp16 2x tree to width 64 + reduce.
                w = FREE // 2
                src = dcq[:, :nb, :]
                while w >= 64:
                    t = tree_pool.tile([JC, QB, w], f16, tag=f"t{w}")
                    nc.vector.tensor_tensor(
                        t[:, :nb, :], src[:, :, :w], src[:, :, w:], Op.min
                    )
                    src = t[:, :nb, :]
                    w //= 2
                nc.vector.tensor_reduce(
                    out=colmin_sb[:, off : off + nb],
                    in_=src,
                    axis=mybir.AxisListType.X,
                    op=Op.min,
                )
                off += nb

            nc.sync.dma_start(colmin_d[:], colmin_sb[:])
            nc.sync.dma_start(rowacc_d[:], racc[len(batches) % 2][:])

    nc.compile()
    return nc


def _augment(pts):
    """pts [N, 3] f32 -> (ahat15 [15, N], bhat15 [15, N]) fp16 hi/lo split.

    ahat = [|a|^2, 1, ax, ay, az]; bhat = [1, |b|^2, -2bx, -2by, -2bz]
    so ahat.bhat = ||a - b||^2.  fp16 split (per column vector v):
    v = v_hi + v_lo + O(2^-22 |v|).  The K=15 layouts
        ahat15 = [a_hi; a_lo; a_hi],  bhat15 = [b_hi; b_hi; b_lo]
    give a_hi.b_hi + a_lo.b_hi + a_hi.b_lo = a.b - a_lo.b_lo - eps.
    """
    pts = np.asarray(pts, dtype=np.float32)
    sq = np.sum(pts * pts, axis=1, dtype=np.float32)
    n = pts.shape[0]
    ahat = np.empty((K, n), dtype=np.float32)
    ahat[0] = sq
    ahat[1] = 1.0
    ahat[2:5] = pts.T
    bhat = np.empty((K, n), dtype=np.float32)
    bhat[0] = 1.0
    bhat[1] = sq
    bhat[2:5] = -2.0 * pts.T

    if MM_DTYPE == "f16":
        dt = np.float16
    else:
        import ml_dtypes

        dt = ml_dtypes.bfloat16
    a_hi = ahat.astype(dt)
    a_lo = (ahat - a_hi.astype(np.float32)).astype(dt)
    b_hi = bhat.astype(dt)
    b_lo = (bhat - b_hi.astype(np.float32)).astype(dt)
    ahat15 = np.concatenate([a_hi, a_lo, a_hi], axis=0)
    bhat15 = np.concatenate([b_hi, b_hi, b_lo], axis=0)
    return ahat15, bhat15


def kernel(state_x, target):
    global LAST_RESULTS
    from concourse.bass_utils import run_bass_kernel_spmd

    state_x = np.asarray(state_x, dtype=np.float32)
    target = np.asarray(target, dtype=np.float32)

    if "nc" not in _CACHE:
        _CACHE["nc"] = _build_nc()
    nc = _CACHE["nc"]

    ahat, _ = _augment(state_x)   # streaming side: state_x
    _, bhat = _augment(target)    # stationary side: target

    in_maps = []
    for k in range(N_CORES):
        sl = slice(k * I_PER_CORE, (k + 1) * I_PER_CORE)
        ab = np.concatenate([ahat[:, sl], bhat], axis=1)
        in_maps.append({"ab_aug": np.ascontiguousarray(ab)})

    res = run_bass_kernel_spmd(nc, in_maps, core_ids=list(range(N_CORES)))
    LAST_RESULTS = res

    # dist2[i] = min_j d(i, j): partition-residue min of the row accumulator.
    dist2 = np.empty(N, dtype=np.float32)
    # dist1[j] = min_i d(i, j): combine per-core partials.
    dist1 = np.full(N, np.inf, dtype=np.float32)
    for k in range(N_CORES):
        out = res.results[k]
        racc = out["rowacc"].astype(np.float32)       # [128, 2048]
        dist2[k * I_PER_CORE : (k + 1) * I_PER_CORE] = racc.min(axis=0)
        colmin = out["colmin"]                        # [128, 128] [p, c]
        dist1 = np.minimum(dist1, colmin.T.reshape(N))

    dist1 = np.maximum(dist1, 0.0)
    dist2 = np.maximum(dist2, 0.0)
    loss = 0.5 * (np.mean(np.sqrt(dist1), dtype=np.float32)
                  + np.mean(np.sqrt(dist2), dtype=np.float32)) * 10.0
    return np.float32(loss)


# revision 33
# speedup vs baseline: 2.8500x; 1.0068x over previous
"""Chamfer loss kernel for Trainium2 (8 NeuronCores, SPMD).

Math: loss = 10 * 0.5 * (mean(sqrt(dist1)) + mean(sqrt(dist2)))
  dist1[j] = min_i ||target_j - state_x_i||^2   (over all state_x)
  dist2[i] = min_j ||state_x_i - target_j||^2   (over all target)

Device strategy (per core k of 8):
  - i (state_x) is sharded: core k owns rows [2048k, 2048(k+1)).
  - j (target) is replicated (stationary matmul operand).
  - Augmented vectors  bhat_j = [1, |b|^2, -2bx, -2by, -2bz],
    ahat_i = [|a|^2, 1, ax, ay, az]  give  bhat_j . ahat_i = d(i, j);
    each side is fp16 hi/lo split (K = 15) so the PE computes the full
    squared-distance tile at near-f32 accuracy.
  - Per group c (128 target points): PE computes the d tile [128 j, 2048 i]
    into PSUM (4 banks, 4x N=512 matmuls).
  - ScalarE (ACT) drains PSUM: relu-clamp + fp16 downcast into SBUF
    (batches of 8 groups in one [128, 8, 2048] tile).
  - VectorE (DVE, the min monopoly) runs two fp16 2x-mode tensor_tensor
    min trees per batch: one folding the batch dim (-> running per-i
    accumulator racc, partition-residue form) and one folding the free
    dim to width 64 + a 1x tensor_reduce (-> exact per-j column mins).
  - Host: partition-residue min, cross-core combine, sqrt/mean epilogue.
"""

import os

import numpy as np

N = 16384
N_CORES = 8
I_PER_CORE = N // N_CORES  # 2048 streaming points per core
JC = 128                   # stationary chunk (output partitions per group)
GROUPS = N // JC           # 128 groups per core
FREE = I_PER_CORE          # 2048 free-dim elements per group
MM_N = 512                 # one PSUM bank of f32 output per matmul
K = 5                      # augmented coordinate count
# fp16 hi/lo split: d = a_hi.b_hi + a_lo.b_hi + a_hi.b_lo (error ~2^-21)
KSPLIT = 3 * K             # contraction dim of the fp16 matmul

# Matmul input dtype: "f16" (hi/lo split, ~2^-21 accurate) or "bf16"
# (hi/lo split, ~2^-15 accurate) — bf16 may stream 2x faster on the PE.
MM_DTYPE = os.environ.get("CHAMFER_MM_DTYPE", "f16")

_CACHE = {}

# Results of the last hardware run (BassKernelResults); test harness reads
# this for exec_time_ns when BASS_TRACE=1.
LAST_RESULTS = None


def _build_nc():
    import concourse.mybir as mybir
    from concourse import bacc
    from concourse.tile import TileContext

    f32 = mybir.dt.float32
    f16 = mybir.dt.float16
    mmdt = f16 if MM_DTYPE == "f16" else mybir.dt.bfloat16
    Op = mybir.AluOpType

    nc = bacc.Bacc(
        "TRN2",
        target_bir_lowering=False,
        debug=False,
        enable_asserts=True,
        num_devices=N_CORES,
    )

    # One input tensor (single DMA → single wait sem on the first matmul):
    # [:, :I_PER_CORE] = streaming ahat slice, [:, I_PER_CORE:] = full bhat.
    ab_aug = nc.dram_tensor(
        "ab_aug", [KSPLIT, I_PER_CORE + N], mmdt, kind="ExternalInput"
    )
    colmin_d = nc.dram_tensor("colmin", [JC, GROUPS], f32, kind="ExternalOutput")
    rowacc_d = nc.dram_tensor("rowacc", [JC, FREE], f16, kind="ExternalOutput")

    QB = 8  # groups per DVE batch (amortizes DVE op init/tail overhead)

    with TileContext(nc) as tc:
        with (
            tc.tile_pool(name="const", bufs=1) as const_pool,
            tc.tile_pool(name="copies", bufs=3) as copy_pool,
            tc.tile_pool(name="tree", bufs=1) as tree_pool,
            tc.tile_pool(name="psum", bufs=2, space="PSUM") as psum_pool,
        ):
            a_rep = const_pool.tile([KSPLIT, I_PER_CORE], mmdt)
            b_rep = const_pool.tile([KSPLIT, N], mmdt)
            # Split input DMAs so group 0's first matmul waits only for a
            # small head slice of each operand.
            nc.sync.dma_start(a_rep[:, :MM_N], ab_aug[:, :MM_N])
            nc.sync.dma_start(a_rep[:, MM_N:], ab_aug[:, MM_N:I_PER_CORE])
            nc.sync.dma_start(
                b_rep[:, :JC], ab_aug[:, I_PER_CORE : I_PER_CORE + JC]
            )
            nc.sync.dma_start(
                b_rep[:, JC : 8 * JC],
                ab_aug[:, I_PER_CORE + JC : I_PER_CORE + 8 * JC],
            )
            nc.sync.dma_start(
                b_rep[:, 8 * JC :], ab_aug[:, I_PER_CORE + 8 * JC :]
            )

            colmin_sb = const_pool.tile([JC, GROUPS], f32)

            racc = [
                const_pool.tile([JC, FREE], f16, name=f"racc{i}") for i in range(2)
            ]

            # Ramp-up batch sizes: DVE work starts after one group instead
            # of a full batch of 8.
            batches = [1, 1, 2, 4, 4, 4] + [QB] * ((GROUPS - 16) // QB)
            assert sum(batches) == GROUPS
            off = 0  # first group of this batch
            for bi, nb in enumerate(batches):
                dcq = copy_pool.tile([JC, QB, FREE], f16, tag="dcq")
                for g in range(nb):
                    c = off + g
                    pt = psum_pool.tile([JC, FREE], f32, tag="pt")
                    for s in range(FREE // MM_N):
                        nc.tensor.matmul(
                            pt[:, s * MM_N : (s + 1) * MM_N],
                            b_rep[:, c * JC : (c + 1) * JC],
                            a_rep[:, s * MM_N : (s + 1) * MM_N],
                            start=True,
                            stop=True,
                        )
                    # ACT drains PSUM: clamp to >=0 + fp16 downcast.
                    nc.scalar.activation(
                        dcq[:, g, :], pt[:], mybir.ActivationFunctionType.Relu
                    )
                # DVE row-accumulate: fold the batch's group slices pairwise
                # (all fp16 2x), then one chain update into racc.
                src = dcq[:, :nb, :]
                m = nb
                while m > 1:
                    t = tree_pool.tile([JC, m // 2, FREE], f16, tag=f"r{m}")
                    nc.vector.tensor_tensor(
                        t[:], src[:, 0 : m // 2, :], src[:, m // 2 : m, :], Op.min
                    )
                    src = t[:]
                    m //= 2
                # src is [JC, 1, FREE] (or the dcq slice when nb == 1)
                if bi == 0:
                    nc.vector.tensor_copy(racc[1][:], dcq[:, 0, :])
                else:
                    nc.vector.tensor_tensor(
                        racc[(bi + 1) % 2][:],
                        racc[bi % 2][:],
                        src[:, 0, :],
                        Op.min,
                    )
                # DVE column-min: batched fp16 2x tree to width 64 + reduce.
                w = FREE // 2
                src = dcq[:, :nb, :]
                while w >= 64:
                    t = tree_pool.tile([JC, QB, w], f16, tag=f"t{w}")
                    nc.vector.tensor_tensor(
                        t[:, :nb, :], src[:, :, :w], src[:, :, w:], Op.min
                    )
                    src = t[:, :nb, :]
                    w //= 2
                nc.vector.tensor_reduce(
                    out=colmin_sb[:, off : off + nb],
                    in_=src,
                    axis=mybir.AxisListType.X,
                    op=Op.min,
                )
                off += nb

            nc.sync.dma_start(colmin_d[:], colmin_sb[:])
            nc.sync.dma_start(rowacc_d[:], racc[len(batches) % 2][:])

    nc.compile()
    return nc


def _augment(pts):
    """pts [N, 3] f32 -> (ahat15 [15, N], bhat15 [15, N]) fp16 hi/lo split.

    ahat = [|a|^2, 1, ax, ay, az]; bhat = [1, |b|^2, -2bx, -2by, -2bz]
    so ahat.bhat = ||a - b||^2.  fp16 split (per column vector v):
    v = v_hi + v_lo + O(2^-22 |v|).  The K=15 layouts
        ahat15 = [a_hi; a_lo; a_hi],  bhat15 = [b_hi; b_hi; b_lo]
    give a_hi.b_hi + a_lo.b_hi + a_hi.b_lo = a.b - a_lo.b_lo - eps.
    """
    pts = np.asarray(pts, dtype=np.float32)
    sq = np.sum(pts * pts, axis=1, dtype=np.float32)
    n = pts.shape[0]
    ahat = np.empty((K, n), dtype=np.float32)
    ahat[0] = sq
    ahat[1] = 1.0
    ahat[2:5] = pts.T
    bhat = np.empty((K, n), dtype=np.float32)
    bhat[0] = 1.0
    bhat[1] = sq
    bhat[2:5] = -2.0 * pts.T

    if MM_DTYPE == "f16":
        dt = np.float16
    else:
        import ml_dtypes

        dt = ml_dtypes.bfloat16
    a_hi = ahat.astype(dt)
    a_lo = (ahat - a_hi.astype(np.float32)).astype(dt)
    b_hi = bhat.astype(dt)
    b_lo = (bhat - b_hi.astype(np.float32)).astype(dt)
    ahat15 = np.concatenate([a_hi, a_lo, a_hi], axis=0)
    bhat15 = np.concatenate([b_hi, b_hi, b_lo], axis=0)
    return ahat15, bhat15


def kernel(state_x, target):
    global LAST_RESULTS
    from concourse.bass_utils import run_bass_kernel_spmd

    state_x = np.asarray(state_x, dtype=np.float32)
    target = np.asarray(target, dtype=np.float32)

    if "nc" not in _CACHE:
        _CACHE["nc"] = _build_nc()
    nc = _CACHE["nc"]

    ahat, _ = _augment(state_x)   # streaming side: state_x
    _, bhat = _augment(target)    # stationary side: target

    in_maps = []
    for k in range(N_CORES):
        sl = slice(k * I_PER_CORE, (k + 1) * I_PER_CORE)
        ab = np.concatenate([ahat[:, sl], bhat], axis=1)
        in_maps.append({"ab_aug": np.ascontiguousarray(ab)})

    res = run_bass_kernel_spmd(nc, in_maps, core_ids=list(range(N_CORES)))
    LAST_RESULTS = res

    # dist2[i] = min_j d(i, j): partition-residue min of the row accumulator.
    dist2 = np.empty(N, dtype=np.float32)
    # dist1[j] = min_i d(i, j): combine per-core partials.
    dist1 = np.full(N, np.inf, dtype=np.float32)
    for k in range(N_CORES):
        out = res.results[k]
        racc = out["rowacc"].astype(np.float32)       # [128, 2048]
        dist2[k * I_PER_CORE : (k + 1) * I_PER_CORE] = racc.min(axis=0)
        colmin = out["colmin"]                        # [128, 128] [p, c]
        dist1 = np.minimum(dist1, colmin.T.reshape(N))

    dist1 = np.maximum(dist1, 0.0)
    dist2 = np.maximum(dist2, 0.0)
    loss = 0.5 * (np.mean(np.sqrt(dist1), dtype=np.float32)
                  + np.mean(np.sqrt(dist2), dtype=np.float32)) * 10.0
    return np.float32(loss)
